# revision 31
# baseline (speedup 1.0000x reference)
# Joint histogram with cubic-B-spline Parzen windows on 8 Trainium2 cores.
#
# Math (matches the reference):
#   u = 64 * (x - min) / (max - min)            in [0, 64]
#   W[p, b] = B3(b - u_p), b = 0..63            (cubic B-spline, support |x|<2)
#   H[b, c] = sum_p Wr[p, b] * Wt[p, c] * 4096  (4096 = 1/EPS^2)
#
# Identity used on-device (avoids piecewise select):
#   B3(x) = relu(2-|x|)^3 / 6 - (2/3) * relu(1-|x|)^3
#
# Sharding: batch dim N=16 -> 2 images per core. Global min/max is computed
# on host (it is a trivial reduction) and folded into a per-partition
# scale/offset tensor, so the device kernel is data-independent and the
# compiled program is reused across calls.
#
# Per-core on-device pipeline, per image pair:
#   phase A (128x512 f32): u = sc*x + off; i = int(u); fr = u - i
#                          (u = i + fr exactly; i, fr stored bf16-exact/small)
#   phase B per group of 64 pixel-columns (tiles 128 x 64bins x 64pix, bf16):
#       d  = iota_b - i - fr        (two TTs; b-major layout keeps the
#                                    broadcast operand inner-stride-1 => 2x mode)
#       s  = |d| - 2                 (TS abs_max/subtract)
#       a  = min(s,0) * -6^(-1/3)    (= relu(2-t)/6^(1/3))
#       b' = max((s+1) * -(2/3)^(1/3), 0)  (= relu(1-t)*(2/3)^(1/3))
#       a2 = a^2, b2 = b'^2          (ACT Square, scalar engine)
#       W  = a2*a - b2*b'            (TTs)  == B3(d)
#   matmul: for each of 64 pixel chunks (128 pixels x 64 bins):
#       H_psum += Wr_chunk^T @ Wt_chunk      (PE, PSUM accumulate)
#   out = H_psum * 4096 -> DRAM

import numpy as np
import ml_dtypes

_STATE: dict = {}

_S6 = float(6.0 ** (-0.5))  # square-scale: (s*a)^2 = a^2/6
_S23 = float((2.0 / 3.0) ** 0.5)  # square-scale: (s*b)^2 = (2/3) b^2


def _split_waits(nc, max_waits=1):
    """Hoist all-but-max_waits sync waits from each instruction onto
    injected same-engine NoOps placed immediately before it (walrus on this
    toolchain rejects >1 sync wait per instruction)."""
    from concourse import mybir

    k = 0
    for f in nc.m.functions:
        for b in f.blocks:
            newl = []
            for inst in b.instructions:
                si = inst.sync_info
                if si is not None and si.on_wait and len(si.on_wait) > max_waits:
                    waits = list(si.on_wait)
                    hoist, keep = waits[:-max_waits], waits[-max_waits:]
                    for w in hoist:
                        nop = mybir.InstNoOp(
                            name=f"{inst.name}-hw{k}", ins=[], outs=[]
                        )
                        k += 1
                        nop.engine = inst.engine
                        nop.sync_info = mybir.SyncInfo(on_wait=[w], on_update=[])
                        newl.append(nop)
                    inst.sync_info = mybir.SyncInfo(
                        on_wait=keep, on_update=list(si.on_update or [])
                    )
                newl.append(inst)
            b.instructions[:] = newl
    return k


def _build_program(repeat=1):
    import contextlib

    import concourse.bass as bass
    import concourse.tile as tile
    from concourse import mybir

    f32 = mybir.dt.float32
    bf16 = mybir.dt.bfloat16
    i32 = mybir.dt.int32
    Alu = mybir.AluOpType
    Act = mybir.ActivationFunctionType

    nc = bass.Bass()
    # xr/xt carry u = 64*(x-min)/(max-min) (normalisation folded in on host
    # during sharding).
    xr_d = nc.declare_dram_parameter("xr", [128, 1024], f32, isOutput=False)
    xt_d = nc.declare_dram_parameter("xt", [128, 1024], f32, isOutput=False)
    io_d = nc.declare_dram_parameter("io", [128, 4096], bf16, isOutput=False)
    h_d = nc.declare_dram_parameter("h", [64, 128], f32, isOutput=True)

    with contextlib.ExitStack() as ctx:
        tc = ctx.enter_context(tile.TileContext(nc))
        const = ctx.enter_context(tc.tile_pool(name="const", bufs=1))
        pxp = ctx.enter_context(tc.tile_pool(name="px", bufs=2))
        pha = ctx.enter_context(tc.tile_pool(name="pha", bufs=2))
        wk = ctx.enter_context(tc.tile_pool(name="wk", bufs=1))
        wk2 = ctx.enter_context(tc.tile_pool(name="wk2", bufs=2))
        wwp = ctx.enter_context(tc.tile_pool(name="ww", bufs=2))
        psp = ctx.enter_context(
            tc.tile_pool(name="ps", bufs=2, space=bass.MemorySpace.PSUM)
        )
        outp = ctx.enter_context(tc.tile_pool(name="outp", bufs=2))

        iota_t = const.tile([128, 4096], bf16, tag="iota")
        nc.sync.dma_start(iota_t[:], io_d[:])
        io3 = iota_t[:].rearrange("p (b j) -> p b j", b=64, j=64)

        rep_ctx = contextlib.ExitStack()
        if repeat > 1:
            rep_ctx.enter_context(tc.For_i(0, repeat, 1))

        for pair in range(2):
            ib_s = []
            fb_s = []
            for side, src in ((0, xr_d), (1, xt_d)):
                u = pxp.tile([128, 512], f32, tag=f"x{side}")
                nc.sync.dma_start(u[:], src[:, pair * 512 : (pair + 1) * 512])
                ii = pha.tile([128, 512], i32, tag=f"ii{side}")
                nc.vector.tensor_copy(ii[:], u[:])
                if32 = pha.tile([128, 512], f32, tag=f"if{side}")
                nc.vector.tensor_copy(if32[:], ii[:])
                fr = pha.tile([128, 512], f32, tag=f"fr{side}")
                nc.vector.tensor_sub(fr[:], u[:], if32[:])
                ib = pha.tile([128, 512], bf16, tag=f"ib{side}")
                nc.vector.tensor_copy(ib[:], if32[:])
                fb = pha.tile([128, 512], bf16, tag=f"fb{side}")
                nc.vector.tensor_copy(fb[:], fr[:])
                ib_s.append(ib)
                fb_s.append(fb)

            hps = psp.tile([128, 128], f32, tag="hps")
            first = True
            JG = 32  # pixel-columns per group
            for g in range(16):
                ab_tiles = []
                for side in range(2):
                    ibc = ib_s[side][:, g * JG : (g + 1) * JG][
                        :, None, :
                    ].to_broadcast((128, 64, JG))
                    fbc = fb_s[side][:, g * JG : (g + 1) * JG][
                        :, None, :
                    ].to_broadcast((128, 64, JG))
                    io3g = io3[:, :, :JG]
                    dl = wk.tile([128, 64, JG], bf16, tag=f"dl{side}")
                    nc.vector.tensor_sub(dl[:], io3g, ibc)
                    d = wk.tile([128, 64, JG], bf16, tag=f"d{side}")
                    nc.vector.tensor_sub(d[:], dl[:], fbc)
                    # t = |d| on the scalar engine
                    t = wk2.tile([128, 64, JG], bf16, tag=f"t{side}")
                    nc.scalar.activation(t[:], d[:], Act.Abs)
                    # a = min(t,2)-2 = -relu(2-t);  b = min(t,1)-1 = -relu(1-t)
                    ta = wk.tile([128, 64, JG], bf16, tag=f"ta{side}")
                    nc.vector.tensor_scalar(
                        ta[:], t[:], 2.0, 2.0, Alu.min, Alu.subtract
                    )
                    tb = wk.tile([128, 64, JG], bf16, tag=f"tb{side}")
                    nc.vector.tensor_scalar(
                        tb[:], t[:], 1.0, 1.0, Alu.min, Alu.subtract
                    )
                    # a2 = (k*a)^2 with k^2 folding the B-spline coefficients:
                    #   a2 = relu(2-t)^2/6,  b2 = (2/3)relu(1-t)^2
                    ta2 = wk2.tile([128, 64, JG], bf16, tag=f"ta2{side}")
                    nc.scalar.activation(ta2[:], ta[:], Act.Square, scale=_S6)
                    tb2 = wk2.tile([128, 64, JG], bf16, tag=f"tb2{side}")
                    nc.scalar.activation(tb2[:], tb[:], Act.Square, scale=_S23)
                    # ab[:,0] = a3 = -relu(2-t)^3/6, ab[:,1] = b3 = -(2/3)relu(1-t)^3
                    # W = b3 - a3; the subtraction is folded into the PE via
                    # bilinear expansion of (b3r-a3r)^T (b3t-a3t).
                    ab = wwp.tile([128, 2, 64, JG], bf16, tag=f"ab{side}")
                    nc.vector.tensor_mul(ab[:, 0], ta2[:], ta[:])
                    nc.vector.tensor_mul(ab[:, 1], tb2[:], tb[:])
                    ab_tiles.append(ab)
                for j in range(JG):
                    nc.tensor.matmul(
                        hps[:],
                        ab_tiles[0][:, :, :, j],
                        ab_tiles[1][:, :, :, j],
                        start=first,
                        stop=(g == 15 and j == JG - 1),
                    )
                    first = False
            # H = AA - AB - BA + BB over the four 64x64 blocks of hps
            # (only one PSUM operand allowed per DVE op: stage AB, BA to SBUF)
            s1 = outp.tile([64, 64], f32, tag="s1")
            nc.scalar.copy(s1[:], hps[:64, 64:])
            s2 = outp.tile([64, 64], f32, tag="s2")
            nc.scalar.copy(s2[:], hps[64:, :64])
            t1 = outp.tile([64, 64], f32, tag="t1")
            nc.vector.tensor_sub(t1[:], hps[:64, :64], s1[:])
            t2 = outp.tile([64, 64], f32, tag="t2")
            nc.vector.tensor_sub(t2[:], hps[64:, 64:], s2[:])
            t3 = outp.tile([64, 64], f32, tag="t3")
            nc.vector.tensor_add(t3[:], t1[:], t2[:])
            ot = outp.tile([64, 64], f32, tag="ot")
            nc.scalar.mul(ot[:], t3[:], 4096.0)
            nc.sync.dma_start(h_d[:, pair * 64 : (pair + 1) * 64], ot[:])
        rep_ctx.close()
    _split_waits(nc)
    return nc


def _get_nc(repeat=1):
    key = ("nc", repeat)
    if key not in _STATE:
        _STATE[key] = _build_program(repeat)
    return _STATE[key]


def _get_runner(repeat=1):
    """Cached jit-compiled 8-core runner (run_bass_via_pjrt rebuilds and
    retraces its jit wrapper on every call; this builds it once)."""
    key = ("runner", repeat)
    if key in _STATE:
        return _STATE[key]
    import jax
    import numpy as _np
    from jax.sharding import Mesh, PartitionSpec
    from jax.experimental.shard_map import shard_map
    from concourse import mybir
    from concourse.bass2jax import (
        _bass_exec_p,
        install_neuronx_cc_hook,
        partition_id_tensor,
    )

    install_neuronx_cc_hook()
    nc = _get_nc(repeat)
    partition_name = (
        nc.partition_id_tensor.name if nc.partition_id_tensor else None
    )
    in_names, out_names, out_avals, zero_outs = [], [], [], []
    for alloc in nc.m.functions[0].allocations:
        if not isinstance(alloc, mybir.MemoryLocationSet):
            continue
        name = alloc.memorylocations[0].name
        if alloc.kind == "ExternalInput":
            if name != partition_name:
                in_names.append(name)
        elif alloc.kind == "ExternalOutput":
            shape = tuple(alloc.tensor_shape)
            dtype = mybir.dt.np(alloc.dtype)
            out_names.append(name)
            out_avals.append(jax.core.ShapedArray(shape, dtype))
            zero_outs.append(_np.zeros(shape, dtype))
    n_params = len(in_names)
    n_outs = len(out_avals)
    all_in_names = list(in_names) + list(out_names)
    if partition_name is not None:
        all_in_names.append(partition_name)
    donate = tuple(range(n_params, n_params + n_outs))

    def _body(*args):
        operands = list(args)
        if partition_name is not None:
            operands.append(partition_id_tensor())
        return tuple(
            _bass_exec_p.bind(
                *operands,
                out_avals=tuple(out_avals),
                in_names=tuple(all_in_names),
                out_names=tuple(out_names),
                lowering_input_output_aliases=(),
                sim_require_finite=True,
                sim_require_nnan=True,
                nc=nc,
            )
        )

    devices = jax.devices()[:8]
    mesh = Mesh(_np.asarray(devices), ("core",))
    sharded = jax.jit(
        shard_map(
            _body,
            mesh=mesh,
            in_specs=(PartitionSpec("core"),) * (n_params + n_outs),
            out_specs=(PartitionSpec("core"),) * n_outs,
            check_rep=False,
        ),
        donate_argnums=donate,
        keep_unused=True,
    )

    def run(in_maps):
        concat_in = [
            _np.concatenate([_np.asarray(m[name]) for m in in_maps], axis=0)
            for name in in_names
        ]
        concat_zero = [
            _np.concatenate([z] * len(in_maps), axis=0) for z in zero_outs
        ]
        outs = sharded(*concat_in, *concat_zero)
        outs = [_np.asarray(o) for o in outs]
        results = []
        for c in range(len(in_maps)):
            res = {}
            for i, name in enumerate(out_names):
                rows = outs[i].shape[0] // len(in_maps)
                res[name] = outs[i][c * rows : (c + 1) * rows]
            results.append(res)
        return results

    _STATE[key] = run
    return run


def _make_in_maps(img_ref, img_tar):
    xr = np.ascontiguousarray(np.asarray(img_ref, dtype=np.float32)).reshape(
        16, 65536
    )
    xt = np.ascontiguousarray(np.asarray(img_tar, dtype=np.float32)).reshape(
        16, 65536
    )
    mnr, mxr = np.float32(xr.min()), np.float32(xr.max())
    mnt, mxt = np.float32(xt.min()), np.float32(xt.max())
    # u = 64*(x-min)/(max-min), matching the reference's f32 normalise + scale
    xr = (xr - mnr) * (np.float32(64.0) / (mxr - mnr))
    xt = (xt - mnt) * (np.float32(64.0) / (mxt - mnt))
    io = np.repeat(np.arange(64, dtype=np.float32), 64)[None, :].repeat(128, axis=0)
    io = io.astype(ml_dtypes.bfloat16)
    in_maps = []
    for c in range(8):
        xrc = np.concatenate(
            [xr[2 * c].reshape(128, 512), xr[2 * c + 1].reshape(128, 512)], axis=1
        )
        xtc = np.concatenate(
            [xt[2 * c].reshape(128, 512), xt[2 * c + 1].reshape(128, 512)], axis=1
        )
        in_maps.append(
            {
                "xr": np.ascontiguousarray(xrc),
                "xt": np.ascontiguousarray(xtc),
                "io": io,
            }
        )
    return in_maps


def _run(in_maps, trace=False, **kwargs):
    from concourse.bass_utils import run_bass_kernel_spmd

    nc = _get_nc()
    return run_bass_kernel_spmd(
        nc, in_maps, core_ids=list(range(8)), trace=trace, **kwargs
    )


def kernel(img_ref, img_tar, bins_ref=None, bins_tar=None):
    in_maps = _make_in_maps(img_ref, img_tar)
    results = _get_runner()(in_maps)
    H = np.empty((16, 64, 64), np.float32)
    for c in range(8):
        h = np.asarray(results[c]["h"], dtype=np.float32)
        H[2 * c] = h[:, :64]
        H[2 * c + 1] = h[:, 64:]
    return H
